# revision 10
# baseline (speedup 1.0000x reference)
"""DenseGeneralAqt inference kernel for Trainium2 (8 NeuronCores).

out = (x @ dequant_int8(qkernel)) * qscale,  x:(2,2048,1024) f32,
qkernel:(1024,4096) int8, qscale:(1,4096) f32 -> out:(2,2048,4096) f32.

Strategy: 2D sharding — 4-way over the flattened token axis (M) x 2-way
over features (N); per core a [1024,1024]x[1024,2048] fp16 GEMM whose
PE-streaming floor (256 matmuls of 512 cycles) dominates, so everything
else is arranged to hide under it. Host marshalling pre-packs all DRAM
operands in on-chip consumption order: xT pair-major fp16
[128kp, pair, kt, 256m], weights kt-major fp16 with the per-channel
scale pre-folded (so there is no on-chip dequant or scale broadcast at
all), output tile-major f32 [32, 128, 512]. The PE is heated with N=512
dummy matmuls (full duty cycle -> the HAM clock gate releases 1.2->2.4
GHz right as the first inputs land). The 8 weight k-tile DMAs are
chained (each waits for the previous transfer) because the shared DMA
queues drain all in-flight transfers fairly — chaining is the only way
to make k-tiles complete in consumption order. xT pairs 1-3 are
deferred behind w-chain progress so their bytes never compete with the
critical path. Sweeps run k-outer across all 8 PSUM banks; PSUM->SBUF
drains are plain copies on the vector engine; the final sweep runs
n-outer and its last chain is split into two 256-wide halves so the
closing drain+store is short. Stores alternate Scalar/Sync rings.
"""

import numpy as np

P = 128
B, S, D, F = 2, 2048, 1024, 4096
N_CORES = 8
MSH, NSH = 4, 2                   # shard grid: 4 m-blocks x 2 n-blocks
M_FULL = B * S                    # 4096 rows
M_CORE = M_FULL // MSH            # 1024 rows per core
N_CORE = F // NSH                 # 2048 cols per core
NT = 512                          # n-tile (one PSUM bank of f32)
WM, WK, WN = M_CORE // P, D // P, N_CORE // NT   # 8, 8, 4
NPAIR = WM // 2                   # 4 m-pair sweeps
MP = M_CORE // NPAIR              # 256 m per pair
NWARM = 5

_CACHE: dict = {}


def _build():
    import concourse.tile as tile
    from concourse import bacc, mybir

    nc = bacc.Bacc("TRN2", target_bir_lowering=False, debug=False)

    xt_dram = nc.dram_tensor("xt", [P, NPAIR, WK, MP], mybir.dt.float16, kind="ExternalInput")
    w_dram = nc.dram_tensor("w", [P, WK, N_CORE], mybir.dt.float16, kind="ExternalInput")
    o_dram = nc.dram_tensor("o", [WM * WN, P, NT], mybir.dt.float32, kind="ExternalOutput")

    with tile.TileContext(nc) as tc:
        with (
            tc.tile_pool(name="w", bufs=1) as wp,
            tc.tile_pool(name="xh", bufs=1) as xhp,
            tc.tile_pool(name="o", bufs=10) as op,
            tc.tile_pool(name="ps", bufs=8, space="PSUM") as pp,
        ):
            # Weights (fp16, scale pre-folded, kt-major): one DMA per
            # k-tile on the Sync ring (the earliest issuer post-barrier),
            # chained so they complete in consumption order.
            w_sb = [
                wp.tile([P, N_CORE], mybir.dt.float16, name=f"w{kt}", tag=f"w{kt}")
                for kt in range(WK)
            ]
            wd = []
            for kt in range(WK):
                d = nc.sync.dma_start(w_sb[kt][:], w_dram[:, kt, :])
                if kt > 0:
                    tile.add_dep_helper(d.ins, wd[-1].ins, reason=f"chain w k{kt}")
                wd.append(d)

            # PE warm-up: N=512 dummy matmuls at full duty so the HAM
            # clock gate releases ~3.4us after the first one issues.
            warm = wp.tile([P, NT], mybir.dt.float16, name="warm", tag="warm")
            nc.vector.memset(warm[:], 0)
            warm_ps = pp.tile([P, NT], mybir.dt.float32, name="warm_ps", tag="ps")
            for _ in range(NWARM):
                nc.tensor.matmul(warm_ps[:], warm[:, 0:P], warm[:])

            # xT shard, pair-major, half-pair chunks on the Scalar ring.
            # Pair 0 flows immediately (needed at stream start); pairs
            # 1-3 are deferred behind w-chain progress so their bytes
            # don't sit ahead of the critical weight k-tiles.
            xh = xhp.tile([P, NPAIR, WK, MP], mybir.dt.float16, name="xh", tag="xh")
            xd = []
            for pr in range(NPAIR):
                for hk in range(2):
                    d = nc.scalar.dma_start(
                        xh[:, pr, 4 * hk:4 * hk + 4, :],
                        xt_dram[:, pr, 4 * hk:4 * hk + 4, :],
                    )
                    xd.append(d)
            tile.add_dep_helper(xd[2].ins, wd[3].ins, reason="defer xh p1")
            tile.add_dep_helper(xd[4].ins, wd[5].ins, reason="defer xh p2")
            tile.add_dep_helper(xd[6].ins, wd[6].ins, reason="defer xh p3")

            st_eng = [nc.scalar, nc.sync]

            def drain(pi, mh, nt, ps_ap, cols):
                mi = pi * 2 + mh
                j = mi * WN + nt
                ot = op.tile([P, NT], mybir.dt.float32, name=f"o{j}_{cols.start}", tag="o")
                nc.vector.tensor_copy(ot[:, cols], ps_ap)
                st_eng[(j + cols.start // NT) % 2].dma_start(
                    o_dram[j, :, cols], ot[:, cols]
                )

            def mm(ps_ap, pi, kt, mh, nslice, first, last):
                nc.tensor.matmul(
                    ps_ap,
                    xh[:, pi, kt, mh * P:(mh + 1) * P],
                    w_sb[kt][:, nslice],
                    start=first,
                    stop=last,
                )

            full = slice(0, NT)
            combos = [(mh, nt) for mh in (0, 1) for nt in range(WN)]
            for pi in range(NPAIR):
                if pi < NPAIR - 1:
                    # k-outer: consume each weight k-tile across all 8
                    # PSUM banks as soon as it lands.
                    ps = {
                        c: pp.tile([P, NT], mybir.dt.float32, name=f"ps{pi}_{c[0]}_{c[1]}", tag="ps")
                        for c in combos
                    }
                    for kt in range(WK):
                        for (mh, nt) in combos:
                            mm(ps[(mh, nt)][:], pi, kt, mh,
                               slice(nt * NT, (nt + 1) * NT), kt == 0, kt == WK - 1)
                    for (mh, nt) in combos:
                        drain(pi, mh, nt, ps[(mh, nt)][:], full)
                else:
                    # Last sweep: n-outer so each bank finishes early and
                    # drains/stores overlap the remaining matmuls; the
                    # final chain is split into two 256-wide halves so
                    # the closing drain+store is half-length.
                    for (mh, nt) in combos[:-1]:
                        ps_t = pp.tile([P, NT], mybir.dt.float32, name=f"ps{pi}_{mh}_{nt}", tag="ps")
                        for kt in range(WK):
                            mm(ps_t[:], pi, kt, mh,
                               slice(nt * NT, (nt + 1) * NT), kt == 0, kt == WK - 1)
                        drain(pi, mh, nt, ps_t[:], full)
                    mh, nt = combos[-1]
                    for half in range(2):
                        cols = slice(half * (NT // 2), (half + 1) * (NT // 2))
                        ps_t = pp.tile([P, NT], mybir.dt.float32, name=f"ps{pi}_{mh}_{nt}_{half}", tag="ps")
                        for kt in range(WK):
                            mm(ps_t[:, cols], pi, kt, mh,
                               slice(nt * NT + cols.start, nt * NT + cols.stop),
                               kt == 0, kt == WK - 1)
                        drain(pi, mh, nt, ps_t[:, cols], cols)

    nc.compile()
    return nc


def _get_nc():
    if "nc" not in _CACHE:
        _CACHE["nc"] = _build()
    return _CACHE["nc"]


def _run(x, qkernel, qscale, trace=False):
    from concourse.bass_utils import run_bass_kernel_spmd

    x = np.asarray(x, dtype=np.float32).reshape(M_FULL, D).astype(np.float16)
    w = np.asarray(qkernel).astype(np.float32)
    s = np.asarray(qscale, dtype=np.float32).reshape(1, F)
    wsc = (w * s).astype(np.float16)                          # fold scale

    in_maps = []
    for c in range(N_CORES):
        mb, nb = c % MSH, c // MSH
        xm = x[mb * M_CORE:(mb + 1) * M_CORE]                  # [1024, 1024]
        # [kp, pair, kt, m']  <-  xm[pr*256+m', kt*128+kp]
        xt = np.ascontiguousarray(
            xm.reshape(NPAIR, MP, WK, P).transpose(3, 0, 2, 1)
        )
        wn = wsc[:, nb * N_CORE:(nb + 1) * N_CORE]             # [1024, 2048]
        wk = np.ascontiguousarray(wn.reshape(WK, P, N_CORE).transpose(1, 0, 2))
        in_maps.append({"xt": xt, "w": wk})
    res = run_bass_kernel_spmd(
        _get_nc(), in_maps, core_ids=list(range(N_CORES)), trace=trace
    )
    out = np.empty((M_FULL, F), dtype=np.float32)
    for c in range(N_CORES):
        mb, nb = c % MSH, c // MSH
        oc = res.results[c]["o"].reshape(WM, WN, P, NT).transpose(0, 2, 1, 3)
        out[mb * M_CORE:(mb + 1) * M_CORE, nb * N_CORE:(nb + 1) * N_CORE] = \
            oc.reshape(M_CORE, N_CORE)
    return out.reshape(B, S, F), res


def kernel(x, qkernel, qscale):
    try:
        out, _ = _run(x, qkernel, qscale, trace=False)
    except Exception:
        # One retry for transient device-side failures.
        out, _ = _run(x, qkernel, qscale, trace=False)
    return out


def kernel_traced(x, qkernel, qscale):
    out, res = _run(x, qkernel, qscale, trace=True)
    return out, res


# revision 38
# speedup vs baseline: 1.2955x; 1.2955x over previous
"""DenseGeneralAqt inference kernel for Trainium2 (8 NeuronCores).

out = (x @ dequant_int8(qkernel)) * qscale,  x:(2,2048,1024) f32,
qkernel:(1024,4096) int8, qscale:(1,4096) f32 -> out:(2,2048,4096) f32.

Strategy: 2D sharding — 4-way over the flattened token axis (M) x 2-way
over features (N); per core a [1024,1024]x[1024,2048] fp16 GEMM whose
PE-streaming floor (256 matmuls of 512 cycles) dominates. The matmuls
are TRANSPOSED: the (dequantized) weight subtile [128k,128n] is the
stationary operand and x [128k,512m] is the moving operand, producing
[128n, 512m] PSUM tiles. This makes the per-channel scale a
per-PARTITION scalar at drain time (an 8KB load instead of a 1MB
broadcast) and, critically, halves the startup DMA demand: a sweep
consumes 128KB of int8 weight + 128KB of fp16 x per k-step (~150 GB/s),
and the first sweep touches only the lower n-half of the weights.
Host marshalling pre-packs DRAM operands in consumption order with
2KB-per-partition contiguous runs. The shared DMA queues drain all
in-flight transfers fairly and a single transfer only reaches ~130
GB/s, so all 16 input chunks ride one ring (Sync) in consumption order
under a windowed chain — window 2 while first-chunk latency matters,
window 4 after for throughput — giving both full rate and ordered
completion. Dequant int8->fp16 runs on the vector engine just ahead of
the PE. N=512 dummy matmuls heat the PE (HAM clock gate 1.2->2.4 GHz)
while the first chunks land. Sweeps run k-outer across all 8 PSUM
banks; drains alternate between the vector and scalar (activation)
engines; the final sweep runs n-outer with its last chain split in two
so the closing drain+store is short. Stores all ride the Sync ring
into a tile-major output layout.
"""

import numpy as np

P = 128
B, S, D, F = 2, 2048, 1024, 4096
N_CORES = 8
MSH, NSH = 4, 2                   # shard grid: 4 m-blocks x 2 n-blocks
M_FULL = B * S                    # 4096 rows
M_CORE = M_FULL // MSH            # 1024 rows per core
N_CORE = F // NSH                 # 2048 cols per core
NT = 512                          # m-tile (one PSUM bank of f32)
WK = D // P                       # 8 k-tiles
MB = M_CORE // NT                 # 2 m-blocks
NH = N_CORE // 2                  # 1024: n-half (weight DMA/cast unit)
NSUB = 8                          # n-subtiles per half (128 each)
NWARM = 9

_CACHE: dict = {}


def _build():
    import concourse.tile as tile
    from concourse import bacc, mybir

    nc = bacc.Bacc("TRN2", target_bir_lowering=False, debug=False)

    xt_dram = nc.dram_tensor("xt", [P, MB, WK, NT], mybir.dt.float16, kind="ExternalInput")
    w_dram = nc.dram_tensor("w", [P, 2, WK, NH], mybir.dt.int8, kind="ExternalInput")
    s_dram = nc.dram_tensor("s", [P, 2 * NSUB], mybir.dt.float32, kind="ExternalInput")
    o_dram = nc.dram_tensor("o", [MB * 2 * NSUB, P, NT], mybir.dt.float32, kind="ExternalOutput")

    with tile.TileContext(nc) as tc:
        with (
            tc.tile_pool(name="wi", bufs=1) as wip,
            tc.tile_pool(name="w", bufs=1) as wp,
            tc.tile_pool(name="qs", bufs=1) as qp,
            tc.tile_pool(name="xh", bufs=1) as xhp,
            tc.tile_pool(name="o", bufs=10) as op,
            tc.tile_pool(name="ps", bufs=8, space="PSUM") as pp,
        ):
            # All 16 input DMAs ride the Sync ring in exact consumption
            # order — w a-half and x m0 kt-pair chunks interleaved, then
            # w b-half, then x m1 — with a window-2 same-engine chain
            # (each waits the transfer two before it). The shared DMA
            # queues drain all in-flight transfers fairly and one
            # transfer only reaches ~130 GB/s, so a 2-deep window gives
            # both full rate (~270 GB/s) and ordered completion.
            w_i8 = [
                wip.tile([P, WK, NH], mybir.dt.int8, name=f"wi{h}", tag=f"wi{h}")
                for h in range(2)
            ]
            xh = xhp.tile([P, MB, WK, NT], mybir.dt.float16, name="xh", tag="xh")

            def wchunk(h, k0, k1):
                return nc.sync.dma_start(
                    w_i8[h][:, k0:k1, :], w_dram[:, h, k0:k1, :]
                )

            def xchunk(mb, k0, k1):
                return nc.sync.dma_start(
                    xh[:, mb, k0:k1, :], xt_dram[:, mb, k0:k1, :]
                )

            # kt-pair chunks in consumption order.
            order = []
            for kp2 in range(4):
                order.append(("w", 0, 2 * kp2, 2 * kp2 + 2))
                order.append(("x", 0, 2 * kp2, 2 * kp2 + 2))
            order += [("w", 1, 2 * kp2, 2 * kp2 + 2) for kp2 in range(4)]
            order += [("x", 1, 2 * kp2, 2 * kp2 + 2) for kp2 in range(4)]
            # Window-2 for the first (latency-critical) chunks, then
            # window-4: deeper queue backlog roughly doubles per-queue
            # descriptor throughput once latency no longer matters.
            dmas = []
            for kind, a, k0, k1 in order:
                d = wchunk(a, k0, k1) if kind == "w" else xchunk(a, k0, k1)
                i = len(dmas)
                if 2 <= i < 4:
                    tile.add_dep_helper(d.ins, dmas[i - 2].ins, reason="dma window2")
                elif i >= 4:
                    tile.add_dep_helper(d.ins, dmas[i - 4].ins, reason="dma window4")
                dmas.append(d)

            # Per-channel scale (per-partition here): tiny [128,16] load
            # on the otherwise-free Scalar ring.
            qs = qp.tile([P, 2 * NSUB], mybir.dt.float32)
            nc.scalar.dma_start(qs[:], s_dram[:, :])

            # PE warm-up: N=512 dummy matmuls at full duty so the HAM
            # clock gate releases ~3.4us after the first one issues.
            warm = wp.tile([P, NT], mybir.dt.float16, name="warm", tag="warm")
            nc.gpsimd.memset(warm[:], 0)
            warm_ps = pp.tile([P, NT], mybir.dt.float32, name="warm_ps", tag="ps")
            for _ in range(NWARM):
                nc.tensor.matmul(warm_ps[:], warm[:, 0:P], warm[:])

            # Dequant int8 -> fp16 on the vector engine, in consumption
            # order: a-half k0 (halved, so the first matmuls' weights are
            # ready earliest) then k1..k7, then the b-half.
            w_sb = [
                wp.tile([P, WK, NH], mybir.dt.float16, name=f"w{h}", tag=f"w{h}")
                for h in range(2)
            ]
            nc.vector.tensor_copy(w_sb[0][:, 0, 0:NH // 2], w_i8[0][:, 0, 0:NH // 2])
            nc.vector.tensor_copy(w_sb[0][:, 0, NH // 2:], w_i8[0][:, 0, NH // 2:])
            for kt in range(1, WK):
                nc.vector.tensor_copy(w_sb[0][:, kt, :], w_i8[0][:, kt, :])
            for kt in range(WK):
                nc.vector.tensor_copy(w_sb[1][:, kt, :], w_i8[1][:, kt, :])

            def drain(j, sub_g, ps_ap, cols, last=False):
                # Drains alternate between the vector engine and the
                # scalar (activation) engine so a sweep's 8 bank drains
                # clear in ~4 serial slots; store issues ride the Sync
                # ring (free once the input chain is done) except the
                # final one, which goes out on GpSimd so its descriptor
                # build is not queued behind the previous store's.
                ot = op.tile([P, NT], mybir.dt.float32, name=f"o{j}_{cols.start}", tag="o")
                sc = qs[:, sub_g:sub_g + 1]
                if (sub_g + cols.start // (NT // 2)) % 2 == 0:
                    nc.vector.tensor_scalar_mul(ot[:, cols], ps_ap, sc)
                else:
                    nc.scalar.mul(ot[:, cols], ps_ap, sc)
                eng = nc.gpsimd if last else nc.sync
                eng.dma_start(o_dram[j, :, cols], ot[:, cols])

            def mm(ps_ap, mb, h, kt, sub, mslice, first, last):
                nc.tensor.matmul(
                    ps_ap,
                    w_sb[h][:, kt, sub * P:(sub + 1) * P],
                    xh[:, mb, kt, mslice],
                    start=first,
                    stop=last,
                )

            full = slice(0, NT)
            # Sweeps: (m-block, n-half); last sweep is (1,1).
            sweeps = [(0, 0), (0, 1), (1, 0), (1, 1)]
            for si, (mb, h) in enumerate(sweeps):
                def jof(sub):
                    return mb * 2 * NSUB + h * NSUB + sub
                if si < len(sweeps) - 1:
                    # k-outer: consume each weight k-tile across all 8
                    # PSUM banks as soon as it is dequantized.
                    ps = {
                        s_: pp.tile([P, NT], mybir.dt.float32, name=f"ps{si}_{s_}", tag="ps")
                        for s_ in range(NSUB)
                    }
                    for kt in range(WK):
                        for s_ in range(NSUB):
                            mm(ps[s_][:], mb, h, kt, s_, full, kt == 0, kt == WK - 1)
                    for s_ in range(NSUB):
                        drain(jof(s_), h * NSUB + s_, ps[s_][:], full)
                else:
                    # Last sweep: n-outer so each bank finishes early and
                    # drains/stores overlap the remaining matmuls; the
                    # final chain is split into two 256-wide halves so
                    # the closing drain+store is half-length.
                    for s_ in range(NSUB - 1):
                        ps_t = pp.tile([P, NT], mybir.dt.float32, name=f"ps{si}_{s_}", tag="ps")
                        for kt in range(WK):
                            mm(ps_t[:], mb, h, kt, s_, full, kt == 0, kt == WK - 1)
                        drain(jof(s_), h * NSUB + s_, ps_t[:], full)
                    s_ = NSUB - 1
                    for half in range(2):
                        cols = slice(half * (NT // 2), (half + 1) * (NT // 2))
                        ps_t = pp.tile([P, NT], mybir.dt.float32, name=f"ps{si}_{s_}_{half}", tag="ps")
                        for kt in range(WK):
                            mm(ps_t[:, cols], mb, h, kt, s_, cols, kt == 0, kt == WK - 1)
                        drain(jof(s_), h * NSUB + s_, ps_t[:, cols], cols,
                              last=(half == 1))

    nc.compile()
    return nc


def _get_nc():
    if "nc" not in _CACHE:
        _CACHE["nc"] = _build()
    return _CACHE["nc"]


def _run(x, qkernel, qscale, trace=False):
    from concourse.bass_utils import run_bass_kernel_spmd

    x = np.asarray(x, dtype=np.float32).reshape(M_FULL, D).astype(np.float16)
    w = np.asarray(qkernel)
    if w.dtype != np.int8:
        w = w.astype(np.int8)
    s = np.asarray(qscale, dtype=np.float32).reshape(1, F)

    in_maps = []
    for c in range(N_CORES):
        mbk, nb = c % MSH, c // MSH
        xm = x[mbk * M_CORE:(mbk + 1) * M_CORE]                # [1024, 1024]
        # [kp, mb, kt, m']  <-  xm[mb*512+m', kt*128+kp]
        xt = np.ascontiguousarray(
            xm.reshape(MB, NT, WK, P).transpose(3, 0, 2, 1)
        )
        wn = w[:, nb * N_CORE:(nb + 1) * N_CORE]               # [1024, 2048]
        # [kp, h, kt, n']  <-  wn[kt*128+kp, h*1024+n']
        wk_ = np.ascontiguousarray(
            wn.reshape(WK, P, 2, NH).transpose(1, 2, 0, 3)
        )
        sn = s[0, nb * N_CORE:(nb + 1) * N_CORE]               # [2048]
        # [n', sub]  <-  sn[sub*128+n']
        sc = np.ascontiguousarray(sn.reshape(2 * NSUB, P).T)
        in_maps.append({"xt": xt, "w": wk_, "s": sc})
    res = run_bass_kernel_spmd(
        _get_nc(), in_maps, core_ids=list(range(N_CORES)), trace=trace
    )
    out = np.empty((M_FULL, F), dtype=np.float32)
    for c in range(N_CORES):
        mbk, nb = c % MSH, c // MSH
        # o[j= mb*16 + sub, n', m'] -> out[mb*512+m', sub*128+n']
        oc = res.results[c]["o"].reshape(MB, 2 * NSUB, P, NT).transpose(0, 3, 1, 2)
        out[mbk * M_CORE:(mbk + 1) * M_CORE, nb * N_CORE:(nb + 1) * N_CORE] = \
            oc.reshape(M_CORE, N_CORE)
    return out.reshape(B, S, F), res


def kernel(x, qkernel, qscale):
    # Retries for transient device-side failures (e.g. a NeuronCore left
    # in a bad state by a previous run).
    for attempt in range(3):
        try:
            out, _ = _run(x, qkernel, qscale, trace=False)
            return out
        except Exception:
            if attempt == 2:
                raise
            _CACHE.clear()
    return None


def kernel_traced(x, qkernel, qscale):
    out, res = _run(x, qkernel, qscale, trace=True)
    return out, res


# revision 39
# speedup vs baseline: 1.3072x; 1.0090x over previous
"""DenseGeneralAqt inference kernel for Trainium2 (8 NeuronCores).

out = (x @ dequant_int8(qkernel)) * qscale,  x:(2,2048,1024) f32,
qkernel:(1024,4096) int8, qscale:(1,4096) f32 -> out:(2,2048,4096) f32.

Strategy: 2D sharding — 4-way over the flattened token axis (M) x 2-way
over features (N); per core a [1024,1024]x[1024,2048] fp16 GEMM whose
PE-streaming floor (256 matmuls of 512 cycles) dominates. The matmuls
are TRANSPOSED: the (dequantized) weight subtile [128k,128n] is the
stationary operand and x [128k,512m] is the moving operand, producing
[128n, 512m] PSUM tiles. This makes the per-channel scale a
per-PARTITION scalar at drain time (an 8KB load instead of a 1MB
broadcast) and, critically, halves the startup DMA demand: a sweep
consumes 128KB of int8 weight + 128KB of fp16 x per k-step (~150 GB/s),
and the first sweep touches only the lower n-half of the weights.
Host marshalling pre-packs DRAM operands in consumption order with
2KB-per-partition contiguous runs. The shared DMA queues drain all
in-flight transfers fairly and a single transfer only reaches ~130
GB/s, so all 16 input chunks ride one ring (Sync) in consumption order
under a windowed chain — window 2 while first-chunk latency matters,
window 4 after for throughput — giving both full rate and ordered
completion. Dequant int8->fp16 runs on the vector engine just ahead of
the PE. N=512 dummy matmuls heat the PE (HAM clock gate 1.2->2.4 GHz)
while the first chunks land. Sweeps run k-outer across all 8 PSUM
banks; drains alternate between the vector and scalar (activation)
engines; the final sweep runs n-outer with its last chain split in two
so the closing drain+store is short. Stores all ride the Sync ring
into a tile-major output layout.
"""

import numpy as np

P = 128
B, S, D, F = 2, 2048, 1024, 4096
N_CORES = 8
MSH, NSH = 4, 2                   # shard grid: 4 m-blocks x 2 n-blocks
M_FULL = B * S                    # 4096 rows
M_CORE = M_FULL // MSH            # 1024 rows per core
N_CORE = F // NSH                 # 2048 cols per core
NT = 512                          # m-tile (one PSUM bank of f32)
WK = D // P                       # 8 k-tiles
MB = M_CORE // NT                 # 2 m-blocks
NH = N_CORE // 2                  # 1024: n-half (weight DMA/cast unit)
NSUB = 8                          # n-subtiles per half (128 each)
NWARM = 9

_CACHE: dict = {}


def _build():
    import concourse.tile as tile
    from concourse import bacc, mybir

    nc = bacc.Bacc("TRN2", target_bir_lowering=False, debug=False)

    xt_dram = nc.dram_tensor("xt", [P, MB, WK, NT], mybir.dt.float16, kind="ExternalInput")
    w_dram = nc.dram_tensor("w", [P, 2, WK, NH], mybir.dt.int8, kind="ExternalInput")
    s_dram = nc.dram_tensor("s", [P, 2 * NSUB], mybir.dt.float32, kind="ExternalInput")
    o_dram = nc.dram_tensor("o", [MB * 2 * NSUB, P, NT], mybir.dt.float32, kind="ExternalOutput")

    with tile.TileContext(nc) as tc:
        with (
            tc.tile_pool(name="wi", bufs=1) as wip,
            tc.tile_pool(name="w", bufs=1) as wp,
            tc.tile_pool(name="qs", bufs=1) as qp,
            tc.tile_pool(name="xh", bufs=1) as xhp,
            tc.tile_pool(name="o", bufs=10) as op,
            tc.tile_pool(name="ps", bufs=8, space="PSUM") as pp,
        ):
            # All 16 input DMAs ride the Sync ring in exact consumption
            # order — w a-half and x m0 kt-pair chunks interleaved, then
            # w b-half, then x m1 — with a window-2 same-engine chain
            # (each waits the transfer two before it). The shared DMA
            # queues drain all in-flight transfers fairly and one
            # transfer only reaches ~130 GB/s, so a 2-deep window gives
            # both full rate (~270 GB/s) and ordered completion.
            w_i8 = [
                wip.tile([P, WK, NH], mybir.dt.int8, name=f"wi{h}", tag=f"wi{h}")
                for h in range(2)
            ]
            xh = xhp.tile([P, MB, WK, NT], mybir.dt.float16, name="xh", tag="xh")

            def wchunk(h, k0, k1):
                return nc.sync.dma_start(
                    w_i8[h][:, k0:k1, :], w_dram[:, h, k0:k1, :]
                )

            def xchunk(mb, k0, k1):
                return nc.sync.dma_start(
                    xh[:, mb, k0:k1, :], xt_dram[:, mb, k0:k1, :]
                )

            # kt-pair chunks in consumption order.
            order = []
            for kp2 in range(4):
                order.append(("w", 0, 2 * kp2, 2 * kp2 + 2))
                order.append(("x", 0, 2 * kp2, 2 * kp2 + 2))
            order += [("w", 1, 2 * kp2, 2 * kp2 + 2) for kp2 in range(4)]
            order += [("x", 1, 2 * kp2, 2 * kp2 + 2) for kp2 in range(4)]
            # Window-2 for the first (latency-critical) chunks, then
            # window-4: deeper queue backlog roughly doubles per-queue
            # descriptor throughput once latency no longer matters.
            dmas = []
            for kind, a, k0, k1 in order:
                d = wchunk(a, k0, k1) if kind == "w" else xchunk(a, k0, k1)
                i = len(dmas)
                if 2 <= i < 4:
                    tile.add_dep_helper(d.ins, dmas[i - 2].ins, reason="dma window2")
                elif i >= 4:
                    tile.add_dep_helper(d.ins, dmas[i - 4].ins, reason="dma window4")
                dmas.append(d)

            # Per-channel scale (per-partition here): tiny [128,16] load
            # on the otherwise-free Scalar ring.
            qs = qp.tile([P, 2 * NSUB], mybir.dt.float32)
            nc.scalar.dma_start(qs[:], s_dram[:, :])

            # PE warm-up: N=512 dummy matmuls at full duty so the HAM
            # clock gate releases ~3.4us after the first one issues.
            warm = wp.tile([P, NT], mybir.dt.float16, name="warm", tag="warm")
            nc.gpsimd.memset(warm[:], 0)
            warm_ps = pp.tile([P, NT], mybir.dt.float32, name="warm_ps", tag="ps")
            for _ in range(NWARM):
                nc.tensor.matmul(warm_ps[:], warm[:, 0:P], warm[:])

            # Dequant int8 -> fp16 on the vector engine, in consumption
            # order: a-half k0 (halved, so the first matmuls' weights are
            # ready earliest) then k1..k7, then the b-half.
            w_sb = [
                wp.tile([P, WK, NH], mybir.dt.float16, name=f"w{h}", tag=f"w{h}")
                for h in range(2)
            ]
            nc.vector.tensor_copy(w_sb[0][:, 0, 0:NH // 2], w_i8[0][:, 0, 0:NH // 2])
            nc.vector.tensor_copy(w_sb[0][:, 0, NH // 2:], w_i8[0][:, 0, NH // 2:])
            for kt in range(1, WK):
                nc.vector.tensor_copy(w_sb[0][:, kt, :], w_i8[0][:, kt, :])
            for kt in range(WK):
                nc.vector.tensor_copy(w_sb[1][:, kt, :], w_i8[1][:, kt, :])

            def drain(j, sub_g, ps_ap, cols):
                # Drains alternate between the vector engine and the
                # scalar (activation) engine so a sweep's 8 bank drains
                # clear in ~4 serial slots; store issues all ride the
                # Sync ring, which is free once the input chain is done.
                ot = op.tile([P, NT], mybir.dt.float32, name=f"o{j}_{cols.start}", tag="o")
                sc = qs[:, sub_g:sub_g + 1]
                if (sub_g + cols.start // (NT // 2)) % 2 == 0:
                    nc.vector.tensor_scalar_mul(ot[:, cols], ps_ap, sc)
                else:
                    nc.scalar.mul(ot[:, cols], ps_ap, sc)
                nc.sync.dma_start(o_dram[j, :, cols], ot[:, cols])

            def mm(ps_ap, mb, h, kt, sub, mslice, first, last):
                nc.tensor.matmul(
                    ps_ap,
                    w_sb[h][:, kt, sub * P:(sub + 1) * P],
                    xh[:, mb, kt, mslice],
                    start=first,
                    stop=last,
                )

            full = slice(0, NT)
            # Sweeps: (m-block, n-half); last sweep is (1,1).
            sweeps = [(0, 0), (0, 1), (1, 0), (1, 1)]
            for si, (mb, h) in enumerate(sweeps):
                def jof(sub):
                    return mb * 2 * NSUB + h * NSUB + sub
                if si < len(sweeps) - 1:
                    # k-outer: consume each weight k-tile across all 8
                    # PSUM banks as soon as it is dequantized.
                    ps = {
                        s_: pp.tile([P, NT], mybir.dt.float32, name=f"ps{si}_{s_}", tag="ps")
                        for s_ in range(NSUB)
                    }
                    for kt in range(WK):
                        for s_ in range(NSUB):
                            mm(ps[s_][:], mb, h, kt, s_, full, kt == 0, kt == WK - 1)
                    for s_ in range(NSUB):
                        drain(jof(s_), h * NSUB + s_, ps[s_][:], full)
                else:
                    # Last sweep: n-outer so each bank finishes early and
                    # drains/stores overlap the remaining matmuls; the
                    # final chain is split into two 256-wide halves so
                    # the closing drain+store is half-length.
                    for s_ in range(NSUB - 1):
                        ps_t = pp.tile([P, NT], mybir.dt.float32, name=f"ps{si}_{s_}", tag="ps")
                        for kt in range(WK):
                            mm(ps_t[:], mb, h, kt, s_, full, kt == 0, kt == WK - 1)
                        drain(jof(s_), h * NSUB + s_, ps_t[:], full)
                    s_ = NSUB - 1
                    for half in range(2):
                        cols = slice(half * (NT // 2), (half + 1) * (NT // 2))
                        ps_t = pp.tile([P, NT], mybir.dt.float32, name=f"ps{si}_{s_}_{half}", tag="ps")
                        for kt in range(WK):
                            mm(ps_t[:, cols], mb, h, kt, s_, cols, kt == 0, kt == WK - 1)
                        drain(jof(s_), h * NSUB + s_, ps_t[:, cols], cols)

    nc.compile()
    return nc


def _get_nc():
    if "nc" not in _CACHE:
        _CACHE["nc"] = _build()
    return _CACHE["nc"]


def _run(x, qkernel, qscale, trace=False):
    from concourse.bass_utils import run_bass_kernel_spmd

    x = np.asarray(x, dtype=np.float32).reshape(M_FULL, D).astype(np.float16)
    w = np.asarray(qkernel)
    if w.dtype != np.int8:
        w = w.astype(np.int8)
    s = np.asarray(qscale, dtype=np.float32).reshape(1, F)

    in_maps = []
    for c in range(N_CORES):
        mbk, nb = c % MSH, c // MSH
        xm = x[mbk * M_CORE:(mbk + 1) * M_CORE]                # [1024, 1024]
        # [kp, mb, kt, m']  <-  xm[mb*512+m', kt*128+kp]
        xt = np.ascontiguousarray(
            xm.reshape(MB, NT, WK, P).transpose(3, 0, 2, 1)
        )
        wn = w[:, nb * N_CORE:(nb + 1) * N_CORE]               # [1024, 2048]
        # [kp, h, kt, n']  <-  wn[kt*128+kp, h*1024+n']
        wk_ = np.ascontiguousarray(
            wn.reshape(WK, P, 2, NH).transpose(1, 2, 0, 3)
        )
        sn = s[0, nb * N_CORE:(nb + 1) * N_CORE]               # [2048]
        # [n', sub]  <-  sn[sub*128+n']
        sc = np.ascontiguousarray(sn.reshape(2 * NSUB, P).T)
        in_maps.append({"xt": xt, "w": wk_, "s": sc})
    res = run_bass_kernel_spmd(
        _get_nc(), in_maps, core_ids=list(range(N_CORES)), trace=trace
    )
    out = np.empty((M_FULL, F), dtype=np.float32)
    for c in range(N_CORES):
        mbk, nb = c % MSH, c // MSH
        # o[j= mb*16 + sub, n', m'] -> out[mb*512+m', sub*128+n']
        oc = res.results[c]["o"].reshape(MB, 2 * NSUB, P, NT).transpose(0, 3, 1, 2)
        out[mbk * M_CORE:(mbk + 1) * M_CORE, nb * N_CORE:(nb + 1) * N_CORE] = \
            oc.reshape(M_CORE, N_CORE)
    return out.reshape(B, S, F), res


def kernel(x, qkernel, qscale):
    # Retries for transient device-side failures (e.g. a NeuronCore left
    # in a bad state by a previous run).
    for attempt in range(3):
        try:
            out, _ = _run(x, qkernel, qscale, trace=False)
            return out
        except Exception:
            if attempt == 2:
                raise
            _CACHE.clear()
    return None


def kernel_traced(x, qkernel, qscale):
    out, res = _run(x, qkernel, qscale, trace=True)
    return out, res
